# revision 1
# baseline (speedup 1.0000x reference)
"""Trainium2 Bass kernel for nn_ODEFunc_90159953478502 (MoE routing, inference path).

Math (see reference):
    logits  = x @ Wg[:256] + (t*Wg[512] + bg)      # zeros kill Wg[256:512]
    w       = softmax(logits, axis=-1)             # [B, E]
    eo_e    = tanh(x @ W1[e] + b1[e]) @ W2[e] + b2[e]
    active_e = any_b(w[b,e] > 0.01)                # always >=1 active:
    out     = sum_e active_e * w[:,e,None] * eo_e  # max softmax >= 1/8 > 0.01

Sharding: expert-parallel. Core e holds the full batch plus only W1[e]/W2[e],
computes m_e * w[:,e,None] * (tanh(x@W1[e]+b1[e]) @ W2[e]) in transposed
layout ([D, B]); the host sums the 8 partial outputs. The b2 rank-1 term
(zero for this problem) would be added host-side from a numpy gating pass.

Device structure per core:
  - x arrives pre-transposed (xT [D, B]) so W1/W2/Wg act as matmul lhsT in
    natural layout (out = lhsT.T @ rhs, contraction on partitions).
  - gating runs in [E, B] layout: logits^T [8, B] chunks via lhsT=Wg_x,
    ACT Exp with fused +gbias (no max-subtract: |logits| <= ~6), column sum
    S and row-select E_e via two tiny lhsT matmuls (ones / onehot).
  - w_e = E_e * reciprocal_approx_fast(S); the active mask (E > 0.01*S,
    reduced over the whole batch, dotted with the onehot) is folded into the
    weight row, which is DRAM-bounced and partition-broadcast to [128, B].
  - mm2 runs on unweighted tanh tiles; since w_e does not depend on h,
    the weighting collapses to one per-drain multiply:
    out^T tile = psum * wb (DVE tensor_tensor, PSUM->SBUF).
  - main matmuls are float32r (2 cycles/row measured) or bf16 (1 cycle/row)
    per _MM_BF16; gating always float32r for weight precision.
"""

import sys

if "/opt/trn_rl_repo" not in sys.path:
    sys.path.insert(0, "/opt/trn_rl_repo")

import numpy as np

_B, _D, _H, _E = 4096, 256, 1024, 8
_NCORES = 8
_CHUNK = 512
_NCH = _B // _CHUNK
_DT = _D // 128   # 2 d-tiles
_HT = _H // 128   # 8 h-tiles
_THRESH = 0.01

_MM_BF16 = False  # main-matmul dtype: False -> float32r, True -> bfloat16

_CACHE = {}


def _build(mm_bf16):
    import concourse.bass as bass
    import concourse.tile as tile
    import concourse.mybir as mybir
    from concourse import bacc
    from contextlib import ExitStack

    F32 = mybir.dt.float32
    F32R = mybir.dt.float32r
    BF16 = mybir.dt.bfloat16
    MMDT = BF16 if mm_bf16 else F32R
    MMIO = BF16 if mm_bf16 else F32  # dram dtype of x/W1/W2
    AF = mybir.ActivationFunctionType
    ALU = mybir.AluOpType
    AX = mybir.AxisListType

    nc = bacc.Bacc("TRN2", target_bir_lowering=False, debug=False)

    XT = nc.declare_dram_parameter("XT", [_D, _B], MMIO, isOutput=False)
    W1E = nc.declare_dram_parameter("W1E", [_D, _H], MMIO, isOutput=False)
    W2E = nc.declare_dram_parameter("W2E", [_H, _D], MMIO, isOutput=False)
    B1E = nc.declare_dram_parameter("B1E", [128, _HT], F32, isOutput=False)
    # gating inputs stay fp32 regardless of main dtype
    XTG = nc.declare_dram_parameter("XTG", [_D, _B], F32, isOutput=False)
    WGX = nc.declare_dram_parameter("WGX", [_D, _E], F32, isOutput=False)
    GB = nc.declare_dram_parameter("GB", [_E, 1], F32, isOutput=False)
    SEL = nc.declare_dram_parameter("SEL", [_E, 2], F32, isOutput=False)
    OUTT = nc.declare_dram_parameter("OUTT", [_D, _B], F32, isOutput=True)

    def bcast(src_ap, n):
        # [1, L] view -> [n, L] partition-broadcast view (stride-0 partitions)
        step, cnt = src_ap.ap[-1]
        return bass.AP(tensor=src_ap.tensor, offset=src_ap.offset, ap=[[0, n], [step, cnt]])

    with tile.TileContext(nc) as tc, ExitStack() as ctx:
        const = ctx.enter_context(tc.tile_pool(name="const", bufs=1))
        dpool = ctx.enter_context(tc.tile_pool(name="dram", bufs=1, space="DRAM"))
        epool = ctx.enter_context(tc.tile_pool(name="epool", bufs=4))
        small = ctx.enter_context(tc.tile_pool(name="small", bufs=4))
        wrp = ctx.enter_context(tc.tile_pool(name="wrp", bufs=8))
        wbp = ctx.enter_context(tc.tile_pool(name="wbp", bufs=8))
        tbp = ctx.enter_context(tc.tile_pool(name="tbp", bufs=3))
        crp = ctx.enter_context(tc.tile_pool(name="crp", bufs=10))
        htp = ctx.enter_context(tc.tile_pool(name="htp", bufs=18))
        op = ctx.enter_context(tc.tile_pool(name="op", bufs=5))
        pgs = ctx.enter_context(tc.tile_pool(name="pgs", bufs=2, space="PSUM"))
        ph = ctx.enter_context(tc.tile_pool(name="ph", bufs=4, space="PSUM"))
        po = ctx.enter_context(tc.tile_pool(name="po", bufs=2, space="PSUM"))

        # ---- inputs, ordered by first use; x chunks split 4-ways for
        # multi-queue latency ----------------------------------------------
        wgx_sb = const.tile([128, _DT * _E], F32R)
        for d in range(_DT):
            nc.sync.dma_start(
                wgx_sb[:, d * _E : (d + 1) * _E],
                WGX.ap()[d * 128 : (d + 1) * 128, :].bitcast(F32R),
            )
        gb_sb = const.tile([_E, 1], F32)
        nc.sync.dma_start(gb_sb[:], GB.ap())
        sel_sb = const.tile([_E, 2], F32R)
        nc.sync.dma_start(sel_sb[:], SEL.ap().bitcast(F32R))
        b1_sb = const.tile([128, _HT], F32)
        nc.sync.dma_start(b1_sb[:], B1E.ap())

        xm = {}
        xg = {}

        def load_x_chunk(c):
            for d in range(_DT):
                t = const.tile([128, _CHUNK], MMDT, tag=f"xm_{d}_{c}")
                for q in range(4):
                    nc.sync.dma_start(
                        t[:, q * 128 : (q + 1) * 128],
                        XT.ap()[
                            d * 128 : (d + 1) * 128,
                            c * _CHUNK + q * 128 : c * _CHUNK + (q + 1) * 128,
                        ].bitcast(MMDT),
                    )
                xm[(d, c)] = t
                if mm_bf16:
                    g = const.tile([128, _CHUNK], F32R, tag=f"xg_{d}_{c}")
                    for q in range(4):
                        nc.sync.dma_start(
                            g[:, q * 128 : (q + 1) * 128],
                            XTG.ap()[
                                d * 128 : (d + 1) * 128,
                                c * _CHUNK + q * 128 : c * _CHUNK + (q + 1) * 128,
                            ].bitcast(F32R),
                        )
                    xg[(d, c)] = g
                else:
                    xg[(d, c)] = t

        w1 = {}
        w2 = {}

        def load_w1():
            for d in range(_DT):
                for hh in range(_HT):
                    t = const.tile([128, 128], MMDT, tag=f"w1_{d}_{hh}")
                    nc.sync.dma_start(
                        t[:],
                        W1E.ap()[
                            d * 128 : (d + 1) * 128, hh * 128 : (hh + 1) * 128
                        ].bitcast(MMDT),
                    )
                    w1[(d, hh)] = t

        def load_w2():
            for hh in range(_HT):
                for d2 in range(_DT):
                    t = const.tile([128, 128], MMDT, tag=f"w2_{hh}_{d2}")
                    nc.sync.dma_start(
                        t[:],
                        W2E.ap()[
                            hh * 128 : (hh + 1) * 128, d2 * 128 : (d2 + 1) * 128
                        ].bitcast(MMDT),
                    )
                    w2[(hh, d2)] = t

        load_x_chunk(0)
        load_w1()
        load_x_chunk(1)
        load_w2()
        for c in range(2, _NCH):
            load_x_chunk(c)

        wrow_d = dpool.tile([1, _B], F32)
        trow_d = dpool.tile([1, _B], F32)
        m_d = dpool.tile([1, 1], F32)

        # ---- gating: all chunks; wb (unmasked) available per chunk ---------
        cr_tiles = []
        wb_tiles = {}
        for c in range(_NCH):
            cs = slice(c * _CHUNK, (c + 1) * _CHUNK)
            psg = pgs.tile([_E, _CHUNK], F32, tag="pg")
            for d in range(_DT):
                nc.tensor.matmul(
                    psg[:], wgx_sb[:, d * _E : (d + 1) * _E], xg[(d, c)][:],
                    start=(d == 0), stop=(d == _DT - 1),
                )
            e_sb = epool.tile([_E, _CHUNK], F32R, tag="e_sb")
            nc.scalar.activation(e_sb[:], psg[:], AF.Exp, bias=gb_sb[:])

            pss_s = pgs.tile([1, _CHUNK], F32, tag="pg")
            nc.tensor.matmul(pss_s[:], sel_sb[:, 0:1], e_sb[:], start=True, stop=True)
            pss_w = pgs.tile([1, _CHUNK], F32, tag="pg")
            nc.tensor.matmul(pss_w[:], sel_sb[:, 1:2], e_sb[:], start=True, stop=True)

            recip = small.tile([1, _CHUNK], F32, tag="recip")
            nc.vector.reciprocal_approx_fast(recip[:], pss_s[0:1, :])
            wu = wrp.tile([1, _CHUNK], F32, tag="wu")
            nc.vector.tensor_tensor(wu[:], pss_w[0:1, :], recip[:], ALU.mult)
            trow = small.tile([1, _CHUNK], F32, tag="trow")
            nc.vector.tensor_scalar_mul(trow[:], pss_s[0:1, :], _THRESH)
            nc.gpsimd.dma_start(trow_d[0:1, cs], trow[:])
            nc.gpsimd.dma_start(wrow_d[0:1, cs], wu[:])

            wb = wbp.tile([128, _CHUNK], F32, tag="wb")
            nc.gpsimd.dma_start(wb[:], bcast(wrow_d[0:1, cs], 128))
            wb_tiles[c] = wb

            tb = tbp.tile([_E, _CHUNK], F32, tag="tb")
            nc.gpsimd.dma_start(tb[:], bcast(trow_d[0:1, cs], _E))
            cmp = tbp.tile([_E, _CHUNK], F32, tag="cmp")
            nc.vector.tensor_tensor(cmp[:], e_sb[:].bitcast(F32), tb[:], ALU.is_gt)
            cr = crp.tile([_E, 1], F32, tag="cr")
            nc.vector.reduce_max(cr[:], cmp[:], axis=AX.X)
            cr_tiles.append(cr)

        # active mask -> m_e (exact 0.0/1.0), applied post-drain
        macc = crp.tile([_E, 1], F32, tag="macc")
        nc.vector.tensor_copy(macc[:], cr_tiles[0][:])
        for c in range(1, _NCH):
            nc.vector.tensor_tensor(macc[:], macc[:], cr_tiles[c][:], ALU.max)
        psm = pgs.tile([1, 1], F32, tag="pg")
        nc.tensor.matmul(
            psm[:], macc[:], sel_sb[:, 1:2].bitcast(F32), start=True, stop=True
        )
        m_sb = small.tile([1, 1], F32, tag="m_sb")
        nc.vector.tensor_copy(m_sb[:], psm[:])
        nc.gpsimd.dma_start(m_d[0:1, 0:1], m_sb[:])
        m_bc = const.tile([128, 1], F32)
        nc.gpsimd.dma_start(m_bc[:], bcast(m_d[0:1, 0:1], 128))

        # ---- main, software-pipelined on PE: mm1(c+1) precedes mm2(c) ------
        ht_by_chunk = {}

        def mm1(c):
            ht_tiles = []
            for hh in range(_HT):
                psh = ph.tile([128, _CHUNK], F32, tag="psh")
                for d in range(_DT):
                    nc.tensor.matmul(
                        psh[:], w1[(d, hh)][:], xm[(d, c)][:],
                        start=(d == 0), stop=(d == _DT - 1),
                    )
                ht = htp.tile([128, _CHUNK], MMDT, tag="ht")
                nc.scalar.activation(
                    ht[:], psh[:], AF.Tanh, bias=b1_sb[:, hh : hh + 1]
                )
                ht_tiles.append(ht)
            ht_by_chunk[c] = ht_tiles

        def mm2(c):
            cs = slice(c * _CHUNK, (c + 1) * _CHUNK)
            ht_tiles = ht_by_chunk.pop(c)
            for d2 in range(_DT):
                pso = po.tile([128, _CHUNK], F32, tag="pso")
                for hh in range(_HT):
                    nc.tensor.matmul(
                        pso[:], w2[(hh, d2)][:], ht_tiles[hh][:],
                        start=(hh == 0), stop=(hh == _HT - 1),
                    )
                osb = op.tile([128, _CHUNK], F32, tag="osb")
                nc.vector.tensor_tensor(osb[:], pso[:], wb_tiles[c][:], ALU.mult)
                nc.vector.tensor_scalar_mul(osb[:], osb[:], m_bc[0:128, 0:1])
                nc.sync.dma_start(OUTT.ap()[d2 * 128 : (d2 + 1) * 128, cs], osb[:])

        mm1(0)
        for c in range(_NCH):
            if c + 1 < _NCH:
                mm1(c + 1)
            mm2(c)

    nc.finalize()
    return nc


def _get_nc():
    key = ("nc", _MM_BF16)
    if key not in _CACHE:
        _CACHE[key] = _build(_MM_BF16)
    return _CACHE[key]


def _make_in_maps(t, x, W1, b1, W2, b2, Wg, bg):
    import ml_dtypes

    mmdt = ml_dtypes.bfloat16 if _MM_BF16 else np.float32
    xT = np.ascontiguousarray(x.T.astype(np.float32))
    xTm = np.ascontiguousarray(xT.astype(mmdt))
    wgx = np.ascontiguousarray(Wg[:_D]).astype(np.float32, copy=False)
    gb = (np.float32(t[0]) * Wg[2 * _D] + bg).astype(np.float32).reshape(_E, 1)
    in_maps = []
    for c in range(_NCORES):
        sel = np.zeros((_E, 2), dtype=np.float32)
        sel[:, 0] = 1.0
        sel[c, 1] = 1.0
        in_maps.append(
            {
                "XT": xTm,
                "XTG": xT,
                "W1E": np.ascontiguousarray(W1[c].astype(mmdt)),
                "W2E": np.ascontiguousarray(W2[c].astype(mmdt)),
                "B1E": np.ascontiguousarray(
                    b1[c].reshape(_HT, 128).T, dtype=np.float32
                ),
                "WGX": wgx,
                "GB": np.ascontiguousarray(gb),
                "SEL": sel,
            }
        )
    return in_maps


def _assemble(results, inputs):
    out = np.zeros((_B, _D), dtype=np.float64)
    for c in range(_NCORES):
        out += results[c]["OUTT"].astype(np.float64).T
    b2 = np.asarray(inputs["b2"])
    if np.any(b2):
        # rank-1 bias term sum_e m_e * w[:,e] b2[e,:] — numpy gating replay
        t, x, Wg, bg = (np.asarray(inputs[k]) for k in ("t", "x", "Wg", "bg"))
        logits = x.astype(np.float64) @ Wg[:_D].astype(np.float64)
        logits += np.float64(t[0]) * Wg[2 * _D].astype(np.float64) + bg
        ex = np.exp(logits - logits.max(axis=1, keepdims=True))
        w = ex / ex.sum(axis=1, keepdims=True)
        active = (w > _THRESH).any(axis=0)
        out += (w * active) @ b2.astype(np.float64)
    return out.astype(np.float32)


def run_on_device(t, x, W1, b1, W2, b2, Wg, bg, trace=False):
    from concourse.bass_utils import run_bass_kernel_spmd

    inputs = dict(t=t, x=x, W1=W1, b1=b1, W2=W2, b2=b2, Wg=Wg, bg=bg)
    in_maps = _make_in_maps(**inputs)
    res = run_bass_kernel_spmd(
        _get_nc(), in_maps, list(range(_NCORES)), trace=trace
    )
    return _assemble(res.results, inputs), res


def kernel(t, x, W1, b1, W2, b2, Wg, bg):
    out, _ = run_on_device(t, x, W1, b1, W2, b2, Wg, bg, trace=False)
    return out



# revision 3
# speedup vs baseline: 1.4795x; 1.4795x over previous
"""Trainium2 Bass kernel for nn_ODEFunc_90159953478502 (MoE routing, inference path).

Math (see reference):
    logits  = x @ Wg[:256] + (t*Wg[512] + bg)      # zeros kill Wg[256:512]
    w       = softmax(logits, axis=-1)             # [B, E]
    eo_e    = tanh(x @ W1[e] + b1[e]) @ W2[e] + b2[e]
    active_e = any_b(w[b,e] > 0.01)
    out     = sum_e active_e * w[:,e,None] * eo_e  # >=1 active always:
                                                   # max softmax >= 1/8 > 0.01

Sharding: expert-parallel. Core e holds the full batch plus only W1[e]/W2[e],
computes w[:,e,None] * (tanh(x@W1[e]+b1[e]) @ W2[e]) in transposed layout
([D, B]) plus a 1-element activity mask m_e = any(w[:,e] > 0.01); the host
sums m_e-masked partial outputs. The b2 rank-1 term (zero here) is added
host-side from a numpy gating replay only when b2 != 0.

Device structure per core (all matmul IO fp16, 1 cycle/row on PE + FWL;
fp32 was ~280ns per [128,128]@[128,512] vs ~215ns fp16):
  - all inputs packed host-side into 5 tensors / 9 total dma_starts.
    Each HWDGE dma_start costs ~700ns serialized on the Sync queue, so
    the old 100+ small DMAs (~82us of trigger time) collapse to ~6us.
  - a few warm-up matmuls on a memset tile ramp the PE HAM clock
    (1.2->2.4GHz) while input DMAs are in flight.
  - gating interleaved per chunk c (B split into 8 chunks of 512):
    logits^T [8,512] via lhsT=wgx (2 d-tiles), ACT Exp with fused +gbias
    (no max-subtract: |logits| <= ~6), then ONE [8,2] sel matmul gives
    S (ones column) and E_e (onehot column) rows; w_e = E_e * recip(S).
    The w row is DRAM-bounced and partition-broadcast to wb [128,512]
    (SWDGE on the GpSimd queue, off the critical path).
  - mask: running max of w_e across chunks, one compare vs 0.01 + reduce
    at the end -> MASK [1,1] output. No mask work on the main path.
  - main pipeline per chunk: mm1 (tanh( x@W1 )) into fp16 ht tiles, mm2
    (@W2) one chunk behind; drain = single DVE mult by wb, fp16 output,
    16 output stores [128,512].
"""

import sys

if "/opt/trn_rl_repo" not in sys.path:
    sys.path.insert(0, "/opt/trn_rl_repo")

import numpy as np

_B, _D, _H, _E = 4096, 256, 1024, 8
_NCORES = 8
_CHUNK = 512
_NCH = _B // _CHUNK
_DT = _D // 128   # 2 d-tiles
_HT = _H // 128   # 8 h-tiles
_THRESH = 0.01
_NWARM = 6        # PE warm-up matmuls while input DMAs land

_CACHE = {}


def _build():
    import concourse.bass as bass
    import concourse.tile as tile
    import concourse.mybir as mybir
    from concourse import bacc
    from contextlib import ExitStack

    F32 = mybir.dt.float32
    F16 = mybir.dt.float16
    AF = mybir.ActivationFunctionType
    ALU = mybir.AluOpType
    AX = mybir.AxisListType

    nc = bacc.Bacc("TRN2", target_bir_lowering=False, debug=False)

    # packed inputs (see _make_in_maps for layouts)
    XP = nc.declare_dram_parameter("XP", [128, _NCH * _DT * _CHUNK], F16, isOutput=False)
    W1P = nc.declare_dram_parameter("W1P", [128, _DT * _H], F16, isOutput=False)
    W2P = nc.declare_dram_parameter("W2P", [128, _HT * _D], F16, isOutput=False)
    CH = nc.declare_dram_parameter("CH", [128, _DT * _E + 2], F16, isOutput=False)
    CF = nc.declare_dram_parameter("CF", [128, 1 + _HT], F32, isOutput=False)
    OUTP = nc.declare_dram_parameter("OUTP", [128, _DT * _B], F16, isOutput=True)
    MASK = nc.declare_dram_parameter("MASK", [1, 1], F32, isOutput=True)

    def bcast(src_ap, n):
        # [1, L] view -> [n, L] partition-broadcast view (stride-0 partitions)
        step, cnt = src_ap.ap[-1]
        return bass.AP(tensor=src_ap.tensor, offset=src_ap.offset, ap=[[0, n], [step, cnt]])

    with tile.TileContext(nc) as tc, ExitStack() as ctx:
        const = ctx.enter_context(tc.tile_pool(name="const", bufs=1))
        dpool = ctx.enter_context(tc.tile_pool(name="dram", bufs=1, space="DRAM"))
        small = ctx.enter_context(tc.tile_pool(name="small", bufs=4))
        wrp = ctx.enter_context(tc.tile_pool(name="wrp", bufs=4))
        wbp = ctx.enter_context(tc.tile_pool(name="wbp", bufs=4))
        htp = ctx.enter_context(tc.tile_pool(name="htp", bufs=18))
        op = ctx.enter_context(tc.tile_pool(name="op", bufs=5))
        pg = ctx.enter_context(tc.tile_pool(name="pg", bufs=2, space="PSUM"))
        ph = ctx.enter_context(tc.tile_pool(name="ph", bufs=4, space="PSUM"))
        po = ctx.enter_context(tc.tile_pool(name="po", bufs=2, space="PSUM"))

        # ---- PE warm-up: memset a junk tile, run matmuls on it ------------
        warm = const.tile([128, _CHUNK], F16)
        nc.vector.memset(warm[:], 0.0)
        for _ in range(_NWARM):
            psw = ph.tile([128, _CHUNK], F32, tag="psh")
            nc.tensor.matmul(psw[:], warm[:, 0:128], warm[:], start=True, stop=True)

        # ---- inputs: few big DMAs, ordered by first use -------------------
        ch_sb = const.tile([128, _DT * _E + 2], F16)
        nc.sync.dma_start(ch_sb[:], CH.ap())
        cf_sb = const.tile([128, 1 + _HT], F32)
        nc.sync.dma_start(cf_sb[:], CF.ap())
        w1 = const.tile([128, _DT * _H], F16)
        nc.sync.dma_start(w1[:], W1P.ap())
        xp = const.tile([128, _NCH * _DT * _CHUNK], F16)
        # per-chunk column blocks [c*1024 : (c+1)*1024); first two alone so
        # gating/mm1 of chunk 0/1 start early
        xsplits = [(0, 1), (1, 2), (2, 4), (4, 6), (6, 8)]
        xload = []
        for lo, hi in xsplits:
            s = slice(lo * _DT * _CHUNK, hi * _DT * _CHUNK)
            xload.append((lo, hi))
            if lo == 2:
                # W2 not needed until mm2(0); slot its load after x[0:2]
                w2 = const.tile([128, _HT * _D], F16)
                nc.sync.dma_start(w2[:], W2P.ap())
            nc.sync.dma_start(xp[:, s], XP.ap()[:, s])

        def xm(d, c):  # [128, 512] fp16 rhs view of x chunk c, d-tile d
            o = c * _DT * _CHUNK + d * _CHUNK
            return xp[:, o : o + _CHUNK]

        def w1t(d, hh):  # [128, 128] lhsT view
            o = d * _H + hh * 128
            return w1[:, o : o + 128]

        def w2t(hh, d2):  # [128, 128] lhsT view
            o = hh * _D + d2 * 128
            return w2[:, o : o + 128]

        wgx = lambda d: ch_sb[:, d * _E : (d + 1) * _E]      # [128, 8]
        sel = ch_sb[0:8, _DT * _E : _DT * _E + 2]            # [8, 2]
        gb = cf_sb[0:8, 0:1]                                 # [8, 1]
        b1c = lambda hh: cf_sb[:, 1 + hh : 2 + hh]           # [128, 1]

        wrow_d = dpool.tile([1, _B], F32)
        rmax = small.tile([1, _CHUNK], F32)

        wb_tiles = {}
        ht_by_chunk = {}

        def gating(c):
            cs = slice(c * _CHUNK, (c + 1) * _CHUNK)
            psg = pg.tile([_E, _CHUNK], F32, tag="pg")
            for d in range(_DT):
                nc.tensor.matmul(psg[:], wgx(d), xm(d, c), start=(d == 0), stop=(d == _DT - 1))
            e_sb = small.tile([_E, _CHUNK], F16, tag="e_sb")
            nc.scalar.activation(e_sb[:], psg[:], AF.Exp, bias=gb)
            # two [1,512] matmuls: PSUM reads must start at partition 0
            pss_s = pg.tile([1, _CHUNK], F32, tag="pg")
            nc.tensor.matmul(pss_s[:], sel[:, 0:1], e_sb[:], start=True, stop=True)
            pss_w = pg.tile([1, _CHUNK], F32, tag="pg")
            nc.tensor.matmul(pss_w[:], sel[:, 1:2], e_sb[:], start=True, stop=True)
            recip = small.tile([1, _CHUNK], F32, tag="recip")
            nc.vector.reciprocal_approx_fast(recip[:], pss_s[0:1, :])
            wu = wrp.tile([1, _CHUNK], F32, tag="wu")
            nc.vector.tensor_tensor(wu[:], pss_w[0:1, :], recip[:], ALU.mult)
            if c == 0:
                nc.vector.tensor_copy(rmax[:], wu[:])
            else:
                nc.vector.tensor_tensor(rmax[:], rmax[:], wu[:], ALU.max)
            nc.gpsimd.dma_start(wrow_d[0:1, cs], wu[:])
            wb = wbp.tile([128, _CHUNK], F32, tag="wb")
            nc.gpsimd.dma_start(wb[:], bcast(wrow_d[0:1, cs], 128))
            wb_tiles[c] = wb

        def mm1(c):
            ht_tiles = []
            for hh in range(_HT):
                psh = ph.tile([128, _CHUNK], F32, tag="psh")
                for d in range(_DT):
                    nc.tensor.matmul(psh[:], w1t(d, hh), xm(d, c), start=(d == 0), stop=(d == _DT - 1))
                ht = htp.tile([128, _CHUNK], F16, tag="ht")
                nc.scalar.activation(ht[:], psh[:], AF.Tanh, bias=b1c(hh))
                ht_tiles.append(ht)
            ht_by_chunk[c] = ht_tiles

        def mm2(c):
            ht_tiles = ht_by_chunk.pop(c)
            for d2 in range(_DT):
                pso = po.tile([128, _CHUNK], F32, tag="pso")
                for hh in range(_HT):
                    nc.tensor.matmul(pso[:], w2t(hh, d2), ht_tiles[hh][:], start=(hh == 0), stop=(hh == _HT - 1))
                osb = op.tile([128, _CHUNK], F16, tag="osb")
                nc.vector.tensor_tensor(osb[:], pso[:], wb_tiles[c][:], ALU.mult)
                o = d2 * _B + c * _CHUNK
                nc.sync.dma_start(OUTP.ap()[:, o : o + _CHUNK], osb[:])

        for c in range(_NCH):
            gating(c)
            mm1(c)
            if c >= 1:
                mm2(c - 1)
        mm2(_NCH - 1)

        # ---- active mask from the running row max -------------------------
        mtmp = small.tile([1, _CHUNK], F32, tag="recip")
        nc.vector.tensor_scalar(out=mtmp[:], in0=rmax[:], scalar1=_THRESH, scalar2=None, op0=ALU.is_gt)
        m_sb = small.tile([1, 1], F32, tag="m_sb")
        nc.vector.reduce_max(m_sb[:], mtmp[:], axis=AX.X)
        nc.sync.dma_start(MASK.ap(), m_sb[:])

    nc.finalize()
    return nc


def _get_nc():
    if "nc" not in _CACHE:
        _CACHE["nc"] = _build()
    return _CACHE["nc"]


def _make_in_maps(t, x, W1, b1, W2, b2, Wg, bg):
    f16 = np.float16
    xT = np.ascontiguousarray(x.T).astype(np.float32, copy=False)
    # x packed chunk-major: XP[p, c*1024 + d*512 + b] = xT[d*128+p, c*512+b]
    xP = np.ascontiguousarray(
        xT.reshape(_DT, 128, _NCH, _CHUNK).transpose(1, 2, 0, 3).reshape(128, -1)
    ).astype(f16)
    wgxP = (
        np.asarray(Wg[: _D], dtype=np.float32)
        .reshape(_DT, 128, _E)
        .transpose(1, 0, 2)
        .reshape(128, _DT * _E)
    )
    gb = (np.float32(t[0]) * Wg[2 * _D] + bg).astype(np.float32).reshape(_E, 1)
    in_maps = []
    for c in range(_NCORES):
        # CH fp16: [wgx (16 cols) | sel (2 cols)]
        chP = np.zeros((128, _DT * _E + 2), dtype=f16)
        chP[:, : _DT * _E] = wgxP.astype(f16)
        chP[0:8, _DT * _E] = 1.0       # ones column -> S
        chP[c, _DT * _E + 1] = 1.0     # onehot column -> E_e
        # CF fp32: [gb (1 col) | b1 tiles (8 cols)]
        cfP = np.zeros((128, 1 + _HT), dtype=np.float32)
        cfP[0:8, 0:1] = gb
        cfP[:, 1:] = np.asarray(b1[c], dtype=np.float32).reshape(_HT, 128).T
        in_maps.append(
            {
                "XP": xP,
                "W1P": np.ascontiguousarray(
                    np.asarray(W1[c], dtype=np.float32)
                    .reshape(_DT, 128, _H)
                    .transpose(1, 0, 2)
                    .reshape(128, _DT * _H)
                ).astype(f16),
                "W2P": np.ascontiguousarray(
                    np.asarray(W2[c], dtype=np.float32)
                    .reshape(_HT, 128, _D)
                    .transpose(1, 0, 2)
                    .reshape(128, _HT * _D)
                ).astype(f16),
                "CH": chP,
                "CF": cfP,
            }
        )
    return in_maps


def _assemble(results, inputs):
    out = np.zeros((_B, _D), dtype=np.float64)
    masks = []
    for c in range(_NCORES):
        m = float(results[c]["MASK"][0, 0]) > 0.5
        masks.append(m)
        if m:
            # OUTP[p, d*4096 + b] -> out[b, d*128+p]
            o = results[c]["OUTP"].astype(np.float64).reshape(128, _DT, _B)
            out += o.transpose(2, 1, 0).reshape(_B, _D)
    # all-inactive fallback is unreachable: softmax max >= 1/E = 0.125 > 0.01
    b2 = np.asarray(inputs["b2"])
    if np.any(b2):
        # rank-1 bias term sum_e m_e * w[:,e] b2[e,:] — numpy gating replay
        t, x, Wg, bg = (np.asarray(inputs[k]) for k in ("t", "x", "Wg", "bg"))
        logits = x.astype(np.float64) @ Wg[:_D].astype(np.float64)
        logits += np.float64(t[0]) * Wg[2 * _D].astype(np.float64) + bg
        ex = np.exp(logits - logits.max(axis=1, keepdims=True))
        w = ex / ex.sum(axis=1, keepdims=True)
        active = (w > _THRESH).any(axis=0)
        out += (w * active) @ b2.astype(np.float64)
    return out.astype(np.float32)


def run_on_device(t, x, W1, b1, W2, b2, Wg, bg, trace=False):
    from concourse.bass_utils import run_bass_kernel_spmd

    inputs = dict(t=t, x=x, W1=W1, b1=b1, W2=W2, b2=b2, Wg=Wg, bg=bg)
    in_maps = _make_in_maps(**inputs)
    res = run_bass_kernel_spmd(
        _get_nc(), in_maps, list(range(_NCORES)), trace=trace
    )
    return _assemble(res.results, inputs), res


def kernel(t, x, W1, b1, W2, b2, Wg, bg):
    out, _ = run_on_device(t, x, W1, b1, W2, b2, Wg, bg, trace=False)
    return out
